# revision 41
# baseline (speedup 1.0000x reference)
"""Trainium2 Bass kernel for nn_CapacityTestMemory (scatter_memory).

reference computation:
    memory  = round-robin circular buffer of enc_hidden rows   (B, M, H)
    q       = query_hidden @ q_w + q_b                         (B, H)
    k       = memory @ k_w + k_b                               (B, M, H)
    raw     = einsum('bh,bmh->bm', q, k) / sqrt(H)             (B, M)
    attn    = softmax over top-8 of raw, 0 elsewhere           (B, M)
    out     = (einsum('bm,bmh->bh', attn, memory) + query) @ out_w + out_b

Exact simplifications:
  *  raw[b,m] = memory[b,m,:] . (k_w @ q[b]) / sqrt(H) + const(b); the
     constant (from k_b) shifts every slot equally -> dropped.
  *  The live memory rows are the contiguous enc positions [L-M, L),
     L = min(2*num_pairs, T-3): the circular buffer collapses to a slice.

v2 performance strategy:
  *  scoring runs on the TensorEngine: enc is uploaded host-transposed in
     fp8 (qk^T is the 128x4 stationary, enc^T chunks are the moving
     operand).  fp8 quarters the dominant HBM traffic; scoring accuracy
     only has to place the true top-8 inside a 32-candidate set (margin
     is ~8 sigma of the fp8 noise).
  *  candidates: per 512-slot window, MAX8 + FIND_INDEX8 on the f32
     score rows; 8 candidates/window x 4 windows = 32 per batch.
  *  exact rescore: the 32 candidate rows are gathered from a full-f32
     enc copy and re-scored on DVE against an f32 qk (f32 q_w/k_w
     prologue), so the final top-8 selection and softmax probabilities
     match the f32 reference to ~1e-6.

Sharding: pure data parallel, batch 32 -> 4 batches per core x 8 cores.
"""

import math
from contextlib import ExitStack

import numpy as np

import concourse.bacc as bacc
import concourse.mybir as mybir
from concourse.bass import IndirectOffsetOnAxis
from concourse.masks import make_identity
from concourse.tile import TileContext
from concourse.bass_utils import run_bass_kernel_spmd

B, T, H = 32, 4096, 512
M = 2048            # memory slots
TOPK = 8
VOCAB = 128
NCORES = 8
BP = B // NCORES    # batches per core
HC = H // 128       # h chunks of 128
NCHUNK = 4          # slot chunks (DMA + scoring granularity)
S = M // NCHUNK     # slots per chunk
NCAND = NCHUNK * 8  # candidates per batch
F32 = mybir.dt.float32
BF16 = mybir.dt.bfloat16
FP8 = mybir.dt.float8e4
I32 = mybir.dt.int32
U32 = mybir.dt.uint32

_CACHE = {}
WARM_MMS = 40
PER_CHUNK_GATHER = False   # sliced indirect DMAs wedge the HW (sim-clean)
PER_CHUNK_RESCORE = False


def _build_kernel():
    nc = bacc.Bacc("TRN2", target_bir_lowering=False, debug=False, num_devices=NCORES)

    # host layouts (see _prepare_in_maps):
    #   encT8[j][32*b+hr][(hb, s)] = enc[b, j*S+s, 32*hb+hr]   (fp8)
    #     -> one matmul with a block-diagonal stationary scores all 4
    #        batches at once: 128-partition contraction = 4 batches x 32 h
    #   enc32 = f32 enc slice, natural [BP, M, H] (gather source only)
    #   qw32  = q_w   "(c p) h -> p (c h)"
    #   kwt32 = k_w^T "(c p) h -> p (c h)"
    #   oww   = out_w "(c p) v -> p (c v)"
    HB = H // 32        # 32-row h blocks in the batched contraction
    encT8 = nc.dram_tensor("encT8", [NCHUNK, 128, HB * S], FP8, kind="ExternalInput")
    enc32 = nc.dram_tensor("enc32", [BP, M, H], F32, kind="ExternalInput")
    query = nc.dram_tensor("query", [BP, H], F32, kind="ExternalInput")
    qw16 = nc.dram_tensor("qw16", [128, HC * H], BF16, kind="ExternalInput")
    kwt16 = nc.dram_tensor("kwt16", [128, HC * H], BF16, kind="ExternalInput")
    qb16d = nc.dram_tensor("qb16", [H], BF16, kind="ExternalInput")
    qw32 = nc.dram_tensor("qw32", [128, HC * H], F32, kind="ExternalInput")
    kwt32 = nc.dram_tensor("kwt32", [128, HC * H], F32, kind="ExternalInput")
    qb = nc.dram_tensor("qb", [H], F32, kind="ExternalInput")
    oww = nc.dram_tensor("oww", [128, HC * VOCAB], F32, kind="ExternalInput")
    ob = nc.dram_tensor("ob", [VOCAB], F32, kind="ExternalInput")
    logits = nc.dram_tensor("logits", [BP, VOCAB], F32, kind="ExternalOutput")

    with TileContext(nc) as tc, ExitStack() as ctx:
        cpool = ctx.enter_context(tc.tile_pool(name="const", bufs=1))
        wpool = ctx.enter_context(tc.tile_pool(name="weights", bufs=1))
        epool = ctx.enter_context(tc.tile_pool(name="enc", bufs=4))
        rpool = ctx.enter_context(tc.tile_pool(name="rep", bufs=1))
        pp_sm = ctx.enter_context(tc.tile_pool(name="ppsm", bufs=1, space="PSUM"))
        pp_acc = ctx.enter_context(tc.tile_pool(name="ppacc", bufs=1, space="PSUM"))
        pp_b = ctx.enter_context(tc.tile_pool(name="ppb", bufs=1, space="PSUM"))
        pp_s = ctx.enter_context(tc.tile_pool(name="pps", bufs=3, space="PSUM"))

        # ---- sync ring: bf16 weights (fast qk for scoring), enc chunks,
        #      then f32 weights (exact qk for the rescore)
        qw16_sb = wpool.tile([128, HC * H], BF16)
        nc.sync.dma_start(out=qw16_sb[:], in_=qw16[:])
        kwt16_sb = wpool.tile([128, HC * H], BF16)
        nc.sync.dma_start(out=kwt16_sb[:], in_=kwt16[:])
        e_sbs = []
        for j in range(NCHUNK):
            e_sb = epool.tile([128, BP * HC * S], FP8, tag="e")
            nc.sync.dma_start(out=e_sb[:], in_=encT8[j])
            e_sbs.append(e_sb)
        qw_sb = wpool.tile([128, HC * H], F32)
        nc.sync.dma_start(out=qw_sb[:], in_=qw32[:])
        kwt_sb = wpool.tile([128, HC * H], F32)
        nc.sync.dma_start(out=kwt_sb[:], in_=kwt32[:])

        # ---- constants / small loads (scalar=ACT HWDGE ring) -------------
        ident4 = cpool.tile([4, 4], F32)
        make_identity(nc, ident4[:])
        ones1_bp = cpool.tile([1, BP], F32)
        nc.vector.memset(ones1_bp[:], 1.0)
        ones16 = cpool.tile([1, BP], BF16)
        nc.vector.memset(ones16[:], 1.0)
        # candidate rows are chunk-major: r = j*32 + b*8 + k
        sel_dram = nc.inline_tensor(
            np.tile(np.kron(np.eye(BP), np.ones((1, 8))), (1, NCHUNK))
            .astype(np.float32), name="sel"
        )
        sel128 = cpool.tile([BP, BP * NCAND], F32)
        nc.scalar.dma_start(out=sel128[:], in_=sel_dram[:])
        blk_dram = nc.inline_tensor(
            np.tile(np.kron(np.eye(BP), np.ones((8, 1))), (NCHUNK, 1))
            .astype(np.float32), name="blk"
        )
        blk32 = cpool.tile([BP * NCAND, BP], F32)
        nc.scalar.dma_start(out=blk32[:], in_=blk_dram[:])
        # SP[c, r] = (c == (r//32)*8 + r%8): spreads probs^T rows onto the
        # chunk-major candidate rows (batch selection handled by blk32)
        rr = np.arange(BP * NCAND)
        sp_dram = nc.inline_tensor(
            (np.arange(NCAND)[:, None] == (rr // 32) * 8 + rr % 8)
            .astype(np.float32), name="spread"
        )
        spmat = cpool.tile([NCAND, BP * NCAND], F32)
        nc.scalar.dma_start(out=spmat[:], in_=sp_dram[:])
        boff_dram = nc.inline_tensor(
            (np.arange(BP, dtype=np.float32) * M)[:, None], name="boff"
        )
        boff = cpool.tile([BP, 1], F32)
        nc.scalar.dma_start(out=boff[:], in_=boff_dram[:])
        # mask01[32b+hr, HB'*4? ] -- block-diagonal mask for the batched
        # stationary: 1 iff partition's batch == column's batch
        mask_dram = nc.inline_tensor(
            (np.arange(128)[:, None] // 32 == np.arange(4 * (H // 32))[None, :] % 4)
            .astype(np.float32), name="mask01"
        )
        mask01 = cpool.tile([128, 4 * (H // 32)], F32)
        nc.scalar.dma_start(out=mask01[:], in_=mask_dram[:])
        # R[hr, 32b+hr'] = (hr == hr'): replicates a [32, *] tile to 4 blocks
        rep_dram = nc.inline_tensor(
            (np.arange(128)[None, :] % 32 == np.arange(32)[:, None])
            .astype(np.float32), name="rep4"
        )
        rep4 = cpool.tile([32, 128], F32)
        nc.scalar.dma_start(out=rep4[:], in_=rep_dram[:])

        query_sb = wpool.tile([BP, H], F32)
        nc.scalar.dma_start(out=query_sb[:], in_=query[:])
        qb_sb = wpool.tile([1, H], F32)
        nc.scalar.dma_start(out=qb_sb[:], in_=qb[None, :])
        qb16_sb = wpool.tile([1, H], BF16)
        nc.scalar.dma_start(out=qb16_sb[:], in_=qb16d[None, :])
        ow_sb = wpool.tile([128, HC * VOCAB], F32)
        nc.scalar.dma_start(out=ow_sb[:], in_=oww[:])
        ob_sb = wpool.tile([1, VOCAB], F32)
        nc.scalar.dma_start(out=ob_sb[:], in_=ob[None, :])

        # pre-warm the ACT exp table so the tail doesn't pay the ~2.7us load
        warm = cpool.tile([1, 1], F32)
        nc.vector.memset(warm[:], 0.0)
        nc.scalar.activation(
            out=warm[:], in_=warm[:], func=mybir.ActivationFunctionType.Exp,
        )
        # PE warm-up: a no-dependency matmul stream spanning the weight-DMA
        # window keeps HAM at K=8/8 so the prologue and scoring run at 2.4GHz
        wsrc = cpool.tile([BP, 256], F32)
        nc.vector.memset(wsrc[:], 1.0)
        if WARM_MMS:
            wps = pp_s.tile([BP, S], F32, tag="score_ps")
            for i in range(WARM_MMS):
                nc.tensor.matmul(
                    out=wps[:, 0:256], lhsT=ident4[:], rhs=wsrc[:],
                    start=(i == 0), stop=(i == WARM_MMS - 1),
                )

        # ---- fast prologue (bf16): approximate qk for the fp8 stationary --
        # scoring only needs candidate-grade accuracy, so this chain runs on
        # the early bf16 weights; the exact f32 chain is emitted after the
        # scoring loop and only feeds the rescore.
        qT_ps = pp_sm.tile([128, 64], F32, tag="small")
        for c in range(HC):
            nc.tensor.transpose(
                out=qT_ps[:, c * BP:(c + 1) * BP],
                in_=query_sb[:, c * 128:(c + 1) * 128],
                identity=ident4[:],
            )
        qT16 = wpool.tile([128, HC * BP], BF16)
        nc.vector.tensor_copy(qT16[:], qT_ps[:, 0:HC * BP])

        qa16_ps = pp_acc.tile([BP, H], F32, tag="acc")
        nc.tensor.matmul(out=qa16_ps[:], lhsT=ones16[:], rhs=qb16_sb[:], start=True, stop=False)
        for c in range(HC):
            nc.tensor.matmul(
                out=qa16_ps[:],
                lhsT=qT16[:, c * BP:(c + 1) * BP],
                rhs=qw16_sb[:, c * H:(c + 1) * H],
                start=False,
                stop=(c == HC - 1),
            )
        qa16_sb = wpool.tile([BP, H], F32)
        nc.vector.tensor_copy(qa16_sb[:], qa16_ps[:])

        qaT16_ps = pp_sm.tile([128, 64], F32, tag="small")
        for c in range(HC):
            nc.tensor.transpose(
                out=qaT16_ps[:, c * BP:(c + 1) * BP],
                in_=qa16_sb[:, c * 128:(c + 1) * 128],
                identity=ident4[:],
            )
        qaT16 = wpool.tile([128, HC * BP], BF16)
        nc.vector.tensor_copy(qaT16[:], qaT16_ps[:, 0:HC * BP])

        qk16_ps = pp_acc.tile([BP, H], F32, tag="acc")
        for c in range(HC):
            nc.tensor.matmul(
                out=qk16_ps[:],
                lhsT=qaT16[:, c * BP:(c + 1) * BP],
                rhs=kwt16_sb[:, c * H:(c + 1) * H],
                start=(c == 0),
                stop=(c == HC - 1),
            )
        qk_us = wpool.tile([BP, H], F32)
        nc.vector.tensor_copy(qk_us[:], qk16_ps[:])

        # block-diagonal fp8 stationary qkB8[32b+hr, 4hb+b'] =
        #   (b==b') * qk_us[b, 32hb+hr], built as mask01 * replicate(pattern)
        # pattern[hr, 4hb+b] = qk_us[b, 32hb+hr]: 32-col transposes of qk_us
        pattern_big = pp_sm.tile([128, 64], F32, tag="small")
        pattern_ps = pattern_big[0:32, :]
        for hb in range(HB):
            nc.tensor.transpose(
                out=pattern_ps[:, hb * BP:(hb + 1) * BP],
                in_=qk_us[:, 32 * hb:32 * (hb + 1)],
                identity=ident4[:],
            )
        pattern = wpool.tile([32, 4 * HB], F32)
        nc.vector.tensor_copy(pattern[:], pattern_ps[:])
        rep_ps = pp_b.tile([128, 4 * HB], F32, tag="rep")
        nc.tensor.matmul(out=rep_ps[:], lhsT=rep4[:], rhs=pattern[:], start=True, stop=True)
        qkB8 = wpool.tile([128, 4 * HB], FP8)
        nc.vector.tensor_tensor(
            out=qkB8[:], in0=rep_ps[:], in1=mask01[:], op=mybir.AluOpType.mult
        )

        # ---- scoring on PE + per-chunk candidates, gather, exact rescore --
        # batched contraction: out[b, s] = sum_hb sum_hr
        #   qkB8[32b+hr, 4hb+b] * enc[b, s, 32hb+hr] -- all 4 rows valid.
        # each chunk's candidate funnel / gather / rescore overlaps the next
        # chunk's DMA + matmuls; only chunk 3's copy of this work is serial.
        scores_row = rpool.tile([BP, M], F32, tag="scores")
        idx_col = rpool.tile([BP * NCAND, 1], I32, tag="idx_col")
        rows_sb = rpool.tile([BP * NCAND, H], F32, tag="rows_sb")
        junk = rpool.tile([BP * NCAND, H], F32, tag="junk")
        s_col = rpool.tile([BP * NCAND, 1], F32, tag="s_col")
        s_row = rpool.tile([BP, NCAND], F32, tag="s_row")
        for j in range(NCHUNK):
            ps = pp_s.tile([BP, S], F32, tag="score_ps")
            for hb in range(HB):
                nc.tensor.matmul(
                    out=ps[:],
                    lhsT=qkB8[:, 4 * hb:4 * (hb + 1)],
                    rhs=e_sbs[j][:, hb * S:(hb + 1) * S],
                    start=(hb == 0),
                    stop=(hb == HB - 1),
                )
            if j % 2 == 0:
                nc.vector.tensor_copy(scores_row[:, j * S:(j + 1) * S], ps[:])
            else:
                nc.scalar.copy(out=scores_row[:, j * S:(j + 1) * S], in_=ps[:])

            v8 = rpool.tile([BP, 8], F32, tag=f"v8_{j}")
            nc.vector.max(out=v8[:], in_=scores_row[:, j * S:(j + 1) * S])
            pos8 = rpool.tile([BP, 8], U32, tag=f"pos8_{j}")
            nc.vector.max_index(
                out=pos8[:], in_max=v8[:], in_values=scores_row[:, j * S:(j + 1) * S]
            )
            idxf = rpool.tile([BP, 8], F32, tag=f"idxf_{j}")
            nc.vector.tensor_copy(idxf[:], pos8[:])
            if j:
                nc.vector.tensor_scalar(
                    out=idxf[:], in0=idxf[:],
                    scalar1=float(j * S), scalar2=None, op0=mybir.AluOpType.add,
                )
            nc.vector.tensor_scalar_add(idxf[:], idxf[:], boff[:, :1])
            idx_i32 = rpool.tile([BP, 8], I32, tag=f"idx_i32_{j}")
            nc.vector.tensor_copy(idx_i32[:], idxf[:])
            nc.scalar.dma_start(
                out=idx_col[j * 32:(j + 1) * 32, :], in_=idx_i32[:]
            )
            if PER_CHUNK_GATHER:
                nc.gpsimd.indirect_dma_start(
                    out=rows_sb[j * 32:(j + 1) * 32, :],
                    out_offset=None,
                    in_=enc32[:].rearrange("b m h -> (b m) h"),
                    in_offset=IndirectOffsetOnAxis(
                        ap=idx_col[j * 32:(j + 1) * 32, 0:1], axis=0
                    ),
                )
            if PER_CHUNK_RESCORE:
                nc.vector.scalar_tensor_tensor(
                    out=junk[j * 32:(j + 1) * 32, :],
                    in0=rows_sb[j * 32:(j + 1) * 32, :],
                    scalar=1.0,
                    in1=qkb128[j * 32:(j + 1) * 32, :],
                    op0=mybir.AluOpType.mult, op1=mybir.AluOpType.mult,
                    accum_out=s_col[j * 32:(j + 1) * 32, :],
                )
                nc.scalar.dma_start(
                    out=s_row[:, j * 8:(j + 1) * 8], in_=s_col[j * 32:(j + 1) * 32, :]
                )

        # ---- exact f32 qk chain (feeds only the rescore; emitted after the
        # scoring loop so its weight-DMA wait never blocks the PE stream) ---
        qT32_ps = pp_sm.tile([128, 64], F32, tag="small")
        for c in range(HC):
            nc.tensor.transpose(
                out=qT32_ps[:, c * BP:(c + 1) * BP],
                in_=query_sb[:, c * 128:(c + 1) * 128],
                identity=ident4[:],
            )
        qT_sb = wpool.tile([128, HC * BP], F32)
        nc.vector.tensor_copy(qT_sb[:], qT32_ps[:, 0:HC * BP])

        qa_ps = pp_acc.tile([BP, H], F32, tag="acc")
        nc.tensor.matmul(out=qa_ps[:], lhsT=ones1_bp[:], rhs=qb_sb[:], start=True, stop=False)
        for c in range(HC):
            nc.tensor.matmul(
                out=qa_ps[:],
                lhsT=qT_sb[:, c * BP:(c + 1) * BP],
                rhs=qw_sb[:, c * H:(c + 1) * H],
                start=False,
                stop=(c == HC - 1),
            )
        qa_sb = wpool.tile([BP, H], F32)
        nc.vector.tensor_copy(qa_sb[:], qa_ps[:])

        qaT32_ps = pp_sm.tile([128, 64], F32, tag="small")
        for c in range(HC):
            nc.tensor.transpose(
                out=qaT32_ps[:, c * BP:(c + 1) * BP],
                in_=qa_sb[:, c * 128:(c + 1) * 128],
                identity=ident4[:],
            )
        qaT_sb = wpool.tile([128, HC * BP], F32)
        nc.vector.tensor_copy(qaT_sb[:], qaT32_ps[:, 0:HC * BP])

        qk_ps = pp_acc.tile([BP, H], F32, tag="acc")
        for c in range(HC):
            nc.tensor.matmul(
                out=qk_ps[:],
                lhsT=qaT_sb[:, c * BP:(c + 1) * BP],
                rhs=kwt_sb[:, c * H:(c + 1) * H],
                start=(c == 0),
                stop=(c == HC - 1),
            )
        qk_sb = wpool.tile([BP, H], F32)
        nc.scalar.mul(out=qk_sb[:], in_=qk_ps[:], mul=1.0 / math.sqrt(H))

        # qkb128[j*32+b*8+k, :] = qk_sb[b, :] for the exact rescore
        qkb_ps = pp_b.tile([BP * NCAND, H], F32, tag="qkb")
        nc.tensor.matmul(out=qkb_ps[:], lhsT=sel128[:], rhs=qk_sb[:], start=True, stop=True)
        qkb128 = wpool.tile([BP * NCAND, H], F32)
        nc.scalar.copy(out=qkb128[:], in_=qkb_ps[:])

        if not PER_CHUNK_GATHER:
            nc.gpsimd.indirect_dma_start(
                out=rows_sb[:],
                out_offset=None,
                in_=enc32[:].rearrange("b m h -> (b m) h"),
                in_offset=IndirectOffsetOnAxis(ap=idx_col[:, 0:1], axis=0),
            )
        if not PER_CHUNK_RESCORE:
            nc.vector.scalar_tensor_tensor(
                out=junk[:], in0=rows_sb[:], scalar=1.0, in1=qkb128[:],
                op0=mybir.AluOpType.mult, op1=mybir.AluOpType.mult,
                accum_out=s_col[:],
            )
            for j in range(NCHUNK):
                nc.scalar.dma_start(
                    out=s_row[:, j * 8:(j + 1) * 8],
                    in_=s_col[j * 32:(j + 1) * 32, :],
                )

        # top-8 of the 32 exact scores; mask = selected
        vals = rpool.tile([BP, 8], F32, tag="vals")
        nc.vector.max(out=vals[:], in_=s_row[:])
        mr = rpool.tile([BP, NCAND], F32, tag="mr")
        nc.vector.match_replace(
            out=mr[:], in_to_replace=vals[:], in_values=s_row[:], imm_value=-1e30
        )
        m01 = rpool.tile([BP, NCAND], F32, tag="m01")
        nc.vector.tensor_scalar(
            out=m01[:], in0=mr[:], scalar1=-1e30, scalar2=None,
            op0=mybir.AluOpType.is_equal,
        )
        e_all = rpool.tile([BP, NCAND], F32, tag="e_all")
        nc.scalar.activation(
            out=e_all[:], in_=s_row[:], func=mybir.ActivationFunctionType.Exp,
        )
        e_sel = rpool.tile([BP, NCAND], F32, tag="e_sel")
        nc.vector.tensor_tensor(
            out=e_sel[:], in0=e_all[:], in1=m01[:], op=mybir.AluOpType.mult
        )
        zsum = rpool.tile([BP, 1], F32, tag="zsum")
        nc.vector.reduce_sum(out=zsum[:], in_=e_sel[:], axis=mybir.AxisListType.X)
        rz = rpool.tile([BP, 1], F32, tag="rz")
        nc.vector.reciprocal(out=rz[:], in_=zsum[:])
        probs = rpool.tile([BP, NCAND], F32, tag="probs")
        nc.vector.tensor_scalar_mul(probs[:], e_sel[:], rz[:, :1])

        # ---- retrieved^T = rows^T @ (blk*probs); logits -------------------
        # W[r, b'] = probs[b', (r//32)*8 + r%8] * (batch(r) == b'), built as
        # (SP^T probs^T) masked by blk32 -- no DMA round trip
        small = pp_sm.tile([128, 64], F32, tag="small")
        pT_ps = small[0:NCAND, 0:BP]
        nc.tensor.transpose(out=pT_ps, in_=probs[:], identity=ident4[:])
        pT_sb = rpool.tile([NCAND, BP], F32, tag="pT_sb")
        nc.scalar.copy(out=pT_sb[:], in_=pT_ps)
        wsp_ps = small[:, BP:2 * BP]
        nc.tensor.matmul(out=wsp_ps, lhsT=spmat[:], rhs=pT_sb[:], start=True, stop=True)
        wmat = rpool.tile([BP * NCAND, BP], F32, tag="wmat")
        nc.vector.tensor_tensor(
            out=wmat[:], in0=wsp_ps, in1=blk32[:], op=mybir.AluOpType.mult
        )
        retT_big = pp_sm.tile([128, 64], F32, tag="small")
        retT_ps = retT_big[:, 0:HC * BP]
        for c in range(HC):
            nc.tensor.matmul(
                out=retT_ps[:, c * BP:(c + 1) * BP],
                lhsT=rows_sb[:, c * 128:(c + 1) * 128],
                rhs=wmat[:],
                start=True,
                stop=True,
            )
        xT_sb = rpool.tile([128, HC * BP], F32, tag="xT_sb")
        nc.vector.tensor_add(out=xT_sb[:], in0=retT_ps, in1=qT_sb[:])

        log_ps = pp_acc.tile([BP, VOCAB], F32, tag="acc")
        nc.tensor.matmul(out=log_ps[:], lhsT=ones1_bp[:], rhs=ob_sb[:], start=True, stop=False)
        for c in range(HC):
            nc.tensor.matmul(
                out=log_ps[:],
                lhsT=xT_sb[:, c * BP:(c + 1) * BP],
                rhs=ow_sb[:, c * VOCAB:(c + 1) * VOCAB],
                start=False,
                stop=(c == HC - 1),
            )
        log_sb = rpool.tile([BP, VOCAB], F32, tag="log_sb")
        nc.scalar.copy(out=log_sb[:], in_=log_ps[:])
        nc.sync.dma_start(out=logits[:], in_=log_sb[:])

    nc.compile()
    return nc


def get_nc():
    if "k" not in _CACHE:
        _CACHE["k"] = _build_kernel()
    return _CACHE["k"]


def _prepare_in_maps(enc_hidden, query_hidden, num_pairs, q_w, q_b, k_w, out_w, out_b):
    import ml_dtypes
    fp8 = ml_dtypes.float8_e4m3

    L = min(2 * int(num_pairs), T - 3)
    n_valid = max(0, min(L, M))
    start = max(0, L - M)

    qw32 = np.ascontiguousarray(
        np.asarray(q_w, dtype=np.float32)
        .reshape(HC, 128, H).transpose(1, 0, 2).reshape(128, HC * H)
    )
    kwt32 = np.ascontiguousarray(
        np.ascontiguousarray(np.asarray(k_w, dtype=np.float32).T)
        .reshape(HC, 128, H).transpose(1, 0, 2).reshape(128, HC * H)
    )
    qb = np.ascontiguousarray(q_b, dtype=np.float32)
    bf16 = ml_dtypes.bfloat16
    qw16 = qw32.astype(bf16)
    kwt16 = kwt32.astype(bf16)
    qb16 = qb.astype(bf16)
    oww = np.ascontiguousarray(
        np.asarray(out_w, dtype=np.float32)
        .reshape(HC, 128, VOCAB).transpose(1, 0, 2).reshape(128, HC * VOCAB)
    )
    ob = np.ascontiguousarray(out_b, dtype=np.float32)

    in_maps = []
    for core in range(NCORES):
        b0 = core * BP
        sl = np.asarray(enc_hidden[b0:b0 + BP, start:start + n_valid, :], dtype=np.float32)
        if n_valid < M:
            pad = np.zeros((BP, M, H), dtype=np.float32)
            pad[:, :n_valid, :] = sl
            sl = pad
        else:
            sl = np.ascontiguousarray(sl)
        # encT8[j, 32b+hr, (hb, s)] = sl[b, j*S+s, 32*hb+hr]
        encT8 = np.ascontiguousarray(
            sl.reshape(BP, NCHUNK, S, H // 32, 32).transpose(1, 0, 4, 3, 2)
            .reshape(NCHUNK, 128, (H // 32) * S)
        ).astype(fp8)
        in_maps.append({
            "encT8": encT8,
            "enc32": sl,
            "query": np.ascontiguousarray(query_hidden[b0:b0 + BP, :], dtype=np.float32),
            "qw16": qw16,
            "kwt16": kwt16,
            "qb16": qb16,
            "qw32": qw32,
            "kwt32": kwt32,
            "qb": qb,
            "oww": oww,
            "ob": ob,
        })
    return in_maps


def kernel(enc_hidden, query_hidden, num_pairs, q_w, q_b, k_w, k_b, out_w, out_b,
           **run_kwargs):
    """Full-input entry point: shards across 8 NeuronCores, returns (B, VOCAB).

    k_b is accepted (to match the reference signature) but unused: it shifts
    every attention score by the same per-batch constant, which affects
    neither the top-k selection nor the softmax probabilities.
    """
    enc_hidden = np.asarray(enc_hidden)
    query_hidden = np.asarray(query_hidden)
    nc = get_nc()
    in_maps = _prepare_in_maps(
        enc_hidden, query_hidden, num_pairs, q_w, q_b, k_w, out_w, out_b
    )
    res = run_bass_kernel_spmd(nc, in_maps, core_ids=list(range(NCORES)), **run_kwargs)
    out = np.concatenate([res.results[c]["logits"] for c in range(NCORES)], axis=0)
    kernel.last_results = res
    return out


# revision 42
# speedup vs baseline: 1.6121x; 1.6121x over previous
"""Trainium2 Bass kernel for nn_CapacityTestMemory (scatter_memory).

reference computation:
    memory  = round-robin circular buffer of enc_hidden rows   (B, M, H)
    q       = query_hidden @ q_w + q_b                         (B, H)
    k       = memory @ k_w + k_b                               (B, M, H)
    raw     = einsum('bh,bmh->bm', q, k) / sqrt(H)             (B, M)
    attn    = softmax over top-8 of raw, 0 elsewhere           (B, M)
    out     = (einsum('bm,bmh->bh', attn, memory) + query) @ out_w + out_b

Exact simplifications:
  *  raw[b,m] = memory[b,m,:] . (k_w @ q[b]) / sqrt(H) + const(b); the
     constant (from k_b) shifts every slot equally -> dropped.
  *  The live memory rows are the contiguous enc positions [L-M, L),
     L = min(2*num_pairs, T-3): the circular buffer collapses to a slice.

v2 performance strategy:
  *  scoring runs on the TensorEngine: enc is uploaded host-transposed in
     fp8 (qk^T is the 128x4 stationary, enc^T chunks are the moving
     operand).  fp8 quarters the dominant HBM traffic; scoring accuracy
     only has to place the true top-8 inside a 32-candidate set (margin
     is ~8 sigma of the fp8 noise).
  *  candidates: per 512-slot window, MAX8 + FIND_INDEX8 on the f32
     score rows; 8 candidates/window x 4 windows = 32 per batch.
  *  exact rescore: the 32 candidate rows are gathered from a full-f32
     enc copy and re-scored on DVE against an f32 qk (f32 q_w/k_w
     prologue), so the final top-8 selection and softmax probabilities
     match the f32 reference to ~1e-6.

Sharding: pure data parallel, batch 32 -> 4 batches per core x 8 cores.
"""

import math
from contextlib import ExitStack

import numpy as np

import concourse.bacc as bacc
import concourse.mybir as mybir
from concourse.bass import IndirectOffsetOnAxis
from concourse.masks import make_identity
from concourse.tile import TileContext
from concourse.bass_utils import run_bass_kernel_spmd

B, T, H = 32, 4096, 512
M = 2048            # memory slots
TOPK = 8
VOCAB = 128
NCORES = 8
BP = B // NCORES    # batches per core
HC = H // 128       # h chunks of 128
NCHUNK = 4          # slot chunks (DMA + scoring granularity)
S = M // NCHUNK     # slots per chunk
NCAND = NCHUNK * 8  # candidates per batch
F32 = mybir.dt.float32
BF16 = mybir.dt.bfloat16
FP8 = mybir.dt.float8e4
I32 = mybir.dt.int32
U32 = mybir.dt.uint32

_CACHE = {}
WARM_MMS = 0  # warm-up streams measured 2-3x slower than modeled; scoring self-warms
PER_CHUNK_GATHER = False   # sliced indirect DMAs wedge the HW (sim-clean)
PER_CHUNK_RESCORE = False


def _build_kernel():
    nc = bacc.Bacc("TRN2", target_bir_lowering=False, debug=False, num_devices=NCORES)

    # host layouts (see _prepare_in_maps):
    #   encT8[j][32*b+hr][(hb, s)] = enc[b, j*S+s, 32*hb+hr]   (fp8)
    #     -> one matmul with a block-diagonal stationary scores all 4
    #        batches at once: 128-partition contraction = 4 batches x 32 h
    #   enc32 = f32 enc slice, natural [BP, M, H] (gather source only)
    #   qw32  = q_w   "(c p) h -> p (c h)"
    #   kwt32 = k_w^T "(c p) h -> p (c h)"
    #   oww   = out_w "(c p) v -> p (c v)"
    HB = H // 32        # 32-row h blocks in the batched contraction
    encT8 = nc.dram_tensor("encT8", [NCHUNK, 128, HB * S], FP8, kind="ExternalInput")
    enc32 = nc.dram_tensor("enc32", [BP, M, H], F32, kind="ExternalInput")
    query = nc.dram_tensor("query", [BP, H], F32, kind="ExternalInput")
    qw16 = nc.dram_tensor("qw16", [128, HC * H], BF16, kind="ExternalInput")
    kwt16 = nc.dram_tensor("kwt16", [128, HC * H], BF16, kind="ExternalInput")
    qb16d = nc.dram_tensor("qb16", [H], BF16, kind="ExternalInput")
    qw32 = nc.dram_tensor("qw32", [128, HC * H], F32, kind="ExternalInput")
    kwt32 = nc.dram_tensor("kwt32", [128, HC * H], F32, kind="ExternalInput")
    qb = nc.dram_tensor("qb", [H], F32, kind="ExternalInput")
    oww = nc.dram_tensor("oww", [128, HC * VOCAB], F32, kind="ExternalInput")
    ob = nc.dram_tensor("ob", [VOCAB], F32, kind="ExternalInput")
    logits = nc.dram_tensor("logits", [BP, VOCAB], F32, kind="ExternalOutput")

    with TileContext(nc) as tc, ExitStack() as ctx:
        cpool = ctx.enter_context(tc.tile_pool(name="const", bufs=1))
        wpool = ctx.enter_context(tc.tile_pool(name="weights", bufs=1))
        epool = ctx.enter_context(tc.tile_pool(name="enc", bufs=4))
        rpool = ctx.enter_context(tc.tile_pool(name="rep", bufs=1))
        pp_sm = ctx.enter_context(tc.tile_pool(name="ppsm", bufs=1, space="PSUM"))
        pp_acc = ctx.enter_context(tc.tile_pool(name="ppacc", bufs=1, space="PSUM"))
        pp_b = ctx.enter_context(tc.tile_pool(name="ppb", bufs=1, space="PSUM"))
        pp_s = ctx.enter_context(tc.tile_pool(name="pps", bufs=3, space="PSUM"))

        # ---- sync ring: bf16 weights (fast qk for scoring), enc chunks,
        #      then f32 weights (exact qk for the rescore)
        qw16_sb = wpool.tile([128, HC * H], BF16)
        nc.sync.dma_start(out=qw16_sb[:], in_=qw16[:])
        kwt16_sb = wpool.tile([128, HC * H], BF16)
        nc.sync.dma_start(out=kwt16_sb[:], in_=kwt16[:])
        e_sbs = []
        for j in range(NCHUNK):
            e_sb = epool.tile([128, BP * HC * S], FP8, tag="e")
            nc.sync.dma_start(out=e_sb[:], in_=encT8[j])
            e_sbs.append(e_sb)
        # ---- constants / small loads (scalar=ACT HWDGE ring) -------------
        ident4 = cpool.tile([4, 4], F32)
        make_identity(nc, ident4[:])
        ones1_bp = cpool.tile([1, BP], F32)
        nc.vector.memset(ones1_bp[:], 1.0)
        ones16 = cpool.tile([1, BP], BF16)
        nc.vector.memset(ones16[:], 1.0)
        # candidate rows are chunk-major: r = j*32 + b*8 + k
        sel_dram = nc.inline_tensor(
            np.tile(np.kron(np.eye(BP), np.ones((1, 8))), (1, NCHUNK))
            .astype(np.float32), name="sel"
        )
        sel128 = cpool.tile([BP, BP * NCAND], F32)
        nc.scalar.dma_start(out=sel128[:], in_=sel_dram[:])
        blk_dram = nc.inline_tensor(
            np.tile(np.kron(np.eye(BP), np.ones((8, 1))), (NCHUNK, 1))
            .astype(np.float32), name="blk"
        )
        blk32 = cpool.tile([BP * NCAND, BP], F32)
        nc.scalar.dma_start(out=blk32[:], in_=blk_dram[:])
        # SP[c, r] = (c == (r//32)*8 + r%8): spreads probs^T rows onto the
        # chunk-major candidate rows (batch selection handled by blk32)
        rr = np.arange(BP * NCAND)
        sp_dram = nc.inline_tensor(
            (np.arange(NCAND)[:, None] == (rr // 32) * 8 + rr % 8)
            .astype(np.float32), name="spread"
        )
        spmat = cpool.tile([NCAND, BP * NCAND], F32)
        nc.scalar.dma_start(out=spmat[:], in_=sp_dram[:])
        boff_dram = nc.inline_tensor(
            (np.arange(BP, dtype=np.float32) * M)[:, None], name="boff"
        )
        boff = cpool.tile([BP, 1], F32)
        nc.scalar.dma_start(out=boff[:], in_=boff_dram[:])
        # mask01[32b+hr, HB'*4? ] -- block-diagonal mask for the batched
        # stationary: 1 iff partition's batch == column's batch
        mask_dram = nc.inline_tensor(
            (np.arange(128)[:, None] // 32 == np.arange(4 * (H // 32))[None, :] % 4)
            .astype(np.float32), name="mask01"
        )
        mask01 = cpool.tile([128, 4 * (H // 32)], F32)
        nc.scalar.dma_start(out=mask01[:], in_=mask_dram[:])
        # R[hr, 32b+hr'] = (hr == hr'): replicates a [32, *] tile to 4 blocks
        rep_dram = nc.inline_tensor(
            (np.arange(128)[None, :] % 32 == np.arange(32)[:, None])
            .astype(np.float32), name="rep4"
        )
        rep4 = cpool.tile([32, 128], F32)
        nc.scalar.dma_start(out=rep4[:], in_=rep_dram[:])

        query_sb = wpool.tile([BP, H], F32)
        nc.scalar.dma_start(out=query_sb[:], in_=query[:])
        qb_sb = wpool.tile([1, H], F32)
        nc.scalar.dma_start(out=qb_sb[:], in_=qb[None, :])
        qb16_sb = wpool.tile([1, H], BF16)
        nc.scalar.dma_start(out=qb16_sb[:], in_=qb16d[None, :])
        ow_sb = wpool.tile([128, HC * VOCAB], F32)
        nc.scalar.dma_start(out=ow_sb[:], in_=oww[:])
        ob_sb = wpool.tile([1, VOCAB], F32)
        nc.scalar.dma_start(out=ob_sb[:], in_=ob[None, :])
        # f32 weights ride the scalar ring concurrently with the enc stream;
        # they only feed the rescore's exact qk chain
        qw_sb = wpool.tile([128, HC * H], F32)
        nc.scalar.dma_start(out=qw_sb[:], in_=qw32[:])
        kwt_sb = wpool.tile([128, HC * H], F32)
        nc.scalar.dma_start(out=kwt_sb[:], in_=kwt32[:])

        # pre-warm the ACT exp table so the tail doesn't pay the ~2.7us load
        warm = cpool.tile([1, 1], F32)
        nc.vector.memset(warm[:], 0.0)
        nc.scalar.activation(
            out=warm[:], in_=warm[:], func=mybir.ActivationFunctionType.Exp,
        )
        # PE warm-up: a no-dependency matmul stream spanning the weight-DMA
        # window keeps HAM at K=8/8 so the prologue and scoring run at 2.4GHz
        wsrc = cpool.tile([BP, 256], F32)
        nc.vector.memset(wsrc[:], 1.0)
        if WARM_MMS:
            wps = pp_s.tile([BP, S], F32, tag="score_ps")
            for i in range(WARM_MMS):
                nc.tensor.matmul(
                    out=wps[:, 0:256], lhsT=ident4[:], rhs=wsrc[:],
                    start=(i == 0), stop=(i == WARM_MMS - 1),
                )

        # ---- fast prologue (bf16): approximate qk for the fp8 stationary --
        # scoring only needs candidate-grade accuracy, so this chain runs on
        # the early bf16 weights; the exact f32 chain is emitted after the
        # scoring loop and only feeds the rescore.
        qT_ps = pp_sm.tile([128, 64], F32, tag="small")
        for c in range(HC):
            nc.tensor.transpose(
                out=qT_ps[:, c * BP:(c + 1) * BP],
                in_=query_sb[:, c * 128:(c + 1) * 128],
                identity=ident4[:],
            )
        qT16 = wpool.tile([128, HC * BP], BF16)
        nc.vector.tensor_copy(qT16[:], qT_ps[:, 0:HC * BP])

        qa16_ps = pp_acc.tile([BP, H], F32, tag="acc")
        nc.tensor.matmul(out=qa16_ps[:], lhsT=ones16[:], rhs=qb16_sb[:], start=True, stop=False)
        for c in range(HC):
            nc.tensor.matmul(
                out=qa16_ps[:],
                lhsT=qT16[:, c * BP:(c + 1) * BP],
                rhs=qw16_sb[:, c * H:(c + 1) * H],
                start=False,
                stop=(c == HC - 1),
            )
        qa16_sb = wpool.tile([BP, H], F32)
        nc.vector.tensor_copy(qa16_sb[:], qa16_ps[:])

        qaT16_ps = pp_sm.tile([128, 64], F32, tag="small")
        for c in range(HC):
            nc.tensor.transpose(
                out=qaT16_ps[:, c * BP:(c + 1) * BP],
                in_=qa16_sb[:, c * 128:(c + 1) * 128],
                identity=ident4[:],
            )
        qaT16 = wpool.tile([128, HC * BP], BF16)
        nc.vector.tensor_copy(qaT16[:], qaT16_ps[:, 0:HC * BP])

        qk16_ps = pp_acc.tile([BP, H], F32, tag="acc")
        for c in range(HC):
            nc.tensor.matmul(
                out=qk16_ps[:],
                lhsT=qaT16[:, c * BP:(c + 1) * BP],
                rhs=kwt16_sb[:, c * H:(c + 1) * H],
                start=(c == 0),
                stop=(c == HC - 1),
            )
        qk_us = wpool.tile([BP, H], F32)
        nc.vector.tensor_copy(qk_us[:], qk16_ps[:])

        # block-diagonal fp8 stationary qkB8[32b+hr, 4hb+b'] =
        #   (b==b') * qk_us[b, 32hb+hr], built as mask01 * replicate(pattern)
        # pattern[hr, 4hb+b] = qk_us[b, 32hb+hr]: 32-col transposes of qk_us
        pattern_big = pp_sm.tile([128, 64], F32, tag="small")
        pattern_ps = pattern_big[0:32, :]
        for hb in range(HB):
            nc.tensor.transpose(
                out=pattern_ps[:, hb * BP:(hb + 1) * BP],
                in_=qk_us[:, 32 * hb:32 * (hb + 1)],
                identity=ident4[:],
            )
        pattern = wpool.tile([32, 4 * HB], F32)
        nc.vector.tensor_copy(pattern[:], pattern_ps[:])
        rep_ps = pp_b.tile([128, 4 * HB], F32, tag="rep")
        nc.tensor.matmul(out=rep_ps[:], lhsT=rep4[:], rhs=pattern[:], start=True, stop=True)
        qkB8 = wpool.tile([128, 4 * HB], FP8)
        nc.vector.tensor_tensor(
            out=qkB8[:], in0=rep_ps[:], in1=mask01[:], op=mybir.AluOpType.mult
        )

        # ---- scoring on PE + per-chunk candidates, gather, exact rescore --
        # batched contraction: out[b, s] = sum_hb sum_hr
        #   qkB8[32b+hr, 4hb+b] * enc[b, s, 32hb+hr] -- all 4 rows valid.
        # each chunk's candidate funnel / gather / rescore overlaps the next
        # chunk's DMA + matmuls; only chunk 3's copy of this work is serial.
        scores_row = rpool.tile([BP, M], F32, tag="scores")
        idx_col = rpool.tile([BP * NCAND, 1], I32, tag="idx_col")
        rows_sb = rpool.tile([BP * NCAND, H], F32, tag="rows_sb")
        junk = rpool.tile([BP * NCAND, H], F32, tag="junk")
        s_col = rpool.tile([BP * NCAND, 1], F32, tag="s_col")
        s_row = rpool.tile([BP, NCAND], F32, tag="s_row")
        for j in range(NCHUNK):
            ps = pp_s.tile([BP, S], F32, tag="score_ps")
            for hb in range(HB):
                nc.tensor.matmul(
                    out=ps[:],
                    lhsT=qkB8[:, 4 * hb:4 * (hb + 1)],
                    rhs=e_sbs[j][:, hb * S:(hb + 1) * S],
                    start=(hb == 0),
                    stop=(hb == HB - 1),
                )
            if j % 2 == 0:
                nc.vector.tensor_copy(scores_row[:, j * S:(j + 1) * S], ps[:])
            else:
                nc.scalar.copy(out=scores_row[:, j * S:(j + 1) * S], in_=ps[:])

            v8 = rpool.tile([BP, 8], F32, tag=f"v8_{j}")
            nc.vector.max(out=v8[:], in_=scores_row[:, j * S:(j + 1) * S])
            pos8 = rpool.tile([BP, 8], U32, tag=f"pos8_{j}")
            nc.vector.max_index(
                out=pos8[:], in_max=v8[:], in_values=scores_row[:, j * S:(j + 1) * S]
            )
            idxf = rpool.tile([BP, 8], F32, tag=f"idxf_{j}")
            nc.vector.tensor_copy(idxf[:], pos8[:])
            if j:
                nc.vector.tensor_scalar(
                    out=idxf[:], in0=idxf[:],
                    scalar1=float(j * S), scalar2=None, op0=mybir.AluOpType.add,
                )
            nc.vector.tensor_scalar_add(idxf[:], idxf[:], boff[:, :1])
            idx_i32 = rpool.tile([BP, 8], I32, tag=f"idx_i32_{j}")
            nc.vector.tensor_copy(idx_i32[:], idxf[:])
            nc.scalar.dma_start(
                out=idx_col[j * 32:(j + 1) * 32, :], in_=idx_i32[:]
            )
            if PER_CHUNK_GATHER:
                nc.gpsimd.indirect_dma_start(
                    out=rows_sb[j * 32:(j + 1) * 32, :],
                    out_offset=None,
                    in_=enc32[:].rearrange("b m h -> (b m) h"),
                    in_offset=IndirectOffsetOnAxis(
                        ap=idx_col[j * 32:(j + 1) * 32, 0:1], axis=0
                    ),
                )
            if PER_CHUNK_RESCORE:
                nc.vector.scalar_tensor_tensor(
                    out=junk[j * 32:(j + 1) * 32, :],
                    in0=rows_sb[j * 32:(j + 1) * 32, :],
                    scalar=1.0,
                    in1=qkb128[j * 32:(j + 1) * 32, :],
                    op0=mybir.AluOpType.mult, op1=mybir.AluOpType.mult,
                    accum_out=s_col[j * 32:(j + 1) * 32, :],
                )
                nc.scalar.dma_start(
                    out=s_row[:, j * 8:(j + 1) * 8], in_=s_col[j * 32:(j + 1) * 32, :]
                )

        # ---- exact f32 qk chain (feeds only the rescore; emitted after the
        # scoring loop so its weight-DMA wait never blocks the PE stream) ---
        qT32_ps = pp_sm.tile([128, 64], F32, tag="small")
        for c in range(HC):
            nc.tensor.transpose(
                out=qT32_ps[:, c * BP:(c + 1) * BP],
                in_=query_sb[:, c * 128:(c + 1) * 128],
                identity=ident4[:],
            )
        qT_sb = wpool.tile([128, HC * BP], F32)
        nc.vector.tensor_copy(qT_sb[:], qT32_ps[:, 0:HC * BP])

        qa_ps = pp_acc.tile([BP, H], F32, tag="acc")
        nc.tensor.matmul(out=qa_ps[:], lhsT=ones1_bp[:], rhs=qb_sb[:], start=True, stop=False)
        for c in range(HC):
            nc.tensor.matmul(
                out=qa_ps[:],
                lhsT=qT_sb[:, c * BP:(c + 1) * BP],
                rhs=qw_sb[:, c * H:(c + 1) * H],
                start=False,
                stop=(c == HC - 1),
            )
        qa_sb = wpool.tile([BP, H], F32)
        nc.vector.tensor_copy(qa_sb[:], qa_ps[:])

        qaT32_ps = pp_sm.tile([128, 64], F32, tag="small")
        for c in range(HC):
            nc.tensor.transpose(
                out=qaT32_ps[:, c * BP:(c + 1) * BP],
                in_=qa_sb[:, c * 128:(c + 1) * 128],
                identity=ident4[:],
            )
        qaT_sb = wpool.tile([128, HC * BP], F32)
        nc.vector.tensor_copy(qaT_sb[:], qaT32_ps[:, 0:HC * BP])

        qk_ps = pp_acc.tile([BP, H], F32, tag="acc")
        for c in range(HC):
            nc.tensor.matmul(
                out=qk_ps[:],
                lhsT=qaT_sb[:, c * BP:(c + 1) * BP],
                rhs=kwt_sb[:, c * H:(c + 1) * H],
                start=(c == 0),
                stop=(c == HC - 1),
            )
        qk_sb = wpool.tile([BP, H], F32)
        nc.scalar.mul(out=qk_sb[:], in_=qk_ps[:], mul=1.0 / math.sqrt(H))

        # qkb128[j*32+b*8+k, :] = qk_sb[b, :] for the exact rescore
        qkb_ps = pp_b.tile([BP * NCAND, H], F32, tag="qkb")
        nc.tensor.matmul(out=qkb_ps[:], lhsT=sel128[:], rhs=qk_sb[:], start=True, stop=True)
        qkb128 = wpool.tile([BP * NCAND, H], F32)
        nc.scalar.copy(out=qkb128[:], in_=qkb_ps[:])

        if not PER_CHUNK_GATHER:
            nc.gpsimd.indirect_dma_start(
                out=rows_sb[:],
                out_offset=None,
                in_=enc32[:].rearrange("b m h -> (b m) h"),
                in_offset=IndirectOffsetOnAxis(ap=idx_col[:, 0:1], axis=0),
            )
        if not PER_CHUNK_RESCORE:
            nc.vector.scalar_tensor_tensor(
                out=junk[:], in0=rows_sb[:], scalar=1.0, in1=qkb128[:],
                op0=mybir.AluOpType.mult, op1=mybir.AluOpType.mult,
                accum_out=s_col[:],
            )
            for j in range(NCHUNK):
                nc.scalar.dma_start(
                    out=s_row[:, j * 8:(j + 1) * 8],
                    in_=s_col[j * 32:(j + 1) * 32, :],
                )

        # top-8 of the 32 exact scores; mask = selected
        vals = rpool.tile([BP, 8], F32, tag="vals")
        nc.vector.max(out=vals[:], in_=s_row[:])
        mr = rpool.tile([BP, NCAND], F32, tag="mr")
        nc.vector.match_replace(
            out=mr[:], in_to_replace=vals[:], in_values=s_row[:], imm_value=-1e30
        )
        m01 = rpool.tile([BP, NCAND], F32, tag="m01")
        nc.vector.tensor_scalar(
            out=m01[:], in0=mr[:], scalar1=-1e30, scalar2=None,
            op0=mybir.AluOpType.is_equal,
        )
        e_all = rpool.tile([BP, NCAND], F32, tag="e_all")
        nc.scalar.activation(
            out=e_all[:], in_=s_row[:], func=mybir.ActivationFunctionType.Exp,
        )
        e_sel = rpool.tile([BP, NCAND], F32, tag="e_sel")
        nc.vector.tensor_tensor(
            out=e_sel[:], in0=e_all[:], in1=m01[:], op=mybir.AluOpType.mult
        )
        zsum = rpool.tile([BP, 1], F32, tag="zsum")
        nc.vector.reduce_sum(out=zsum[:], in_=e_sel[:], axis=mybir.AxisListType.X)
        rz = rpool.tile([BP, 1], F32, tag="rz")
        nc.vector.reciprocal(out=rz[:], in_=zsum[:])
        probs = rpool.tile([BP, NCAND], F32, tag="probs")
        nc.vector.tensor_scalar_mul(probs[:], e_sel[:], rz[:, :1])

        # ---- retrieved^T = rows^T @ (blk*probs); logits -------------------
        # W[r, b'] = probs[b', (r//32)*8 + r%8] * (batch(r) == b'), built as
        # (SP^T probs^T) masked by blk32 -- no DMA round trip
        small = pp_sm.tile([128, 64], F32, tag="small")
        pT_ps = small[0:NCAND, 0:BP]
        nc.tensor.transpose(out=pT_ps, in_=probs[:], identity=ident4[:])
        pT_sb = rpool.tile([NCAND, BP], F32, tag="pT_sb")
        nc.scalar.copy(out=pT_sb[:], in_=pT_ps)
        wsp_ps = small[:, BP:2 * BP]
        nc.tensor.matmul(out=wsp_ps, lhsT=spmat[:], rhs=pT_sb[:], start=True, stop=True)
        wmat = rpool.tile([BP * NCAND, BP], F32, tag="wmat")
        nc.vector.tensor_tensor(
            out=wmat[:], in0=wsp_ps, in1=blk32[:], op=mybir.AluOpType.mult
        )
        retT_big = pp_sm.tile([128, 64], F32, tag="small")
        retT_ps = retT_big[:, 0:HC * BP]
        for c in range(HC):
            nc.tensor.matmul(
                out=retT_ps[:, c * BP:(c + 1) * BP],
                lhsT=rows_sb[:, c * 128:(c + 1) * 128],
                rhs=wmat[:],
                start=True,
                stop=True,
            )
        xT_sb = rpool.tile([128, HC * BP], F32, tag="xT_sb")
        nc.vector.tensor_add(out=xT_sb[:], in0=retT_ps, in1=qT_sb[:])

        log_ps = pp_acc.tile([BP, VOCAB], F32, tag="acc")
        nc.tensor.matmul(out=log_ps[:], lhsT=ones1_bp[:], rhs=ob_sb[:], start=True, stop=False)
        for c in range(HC):
            nc.tensor.matmul(
                out=log_ps[:],
                lhsT=xT_sb[:, c * BP:(c + 1) * BP],
                rhs=ow_sb[:, c * VOCAB:(c + 1) * VOCAB],
                start=False,
                stop=(c == HC - 1),
            )
        log_sb = rpool.tile([BP, VOCAB], F32, tag="log_sb")
        nc.scalar.copy(out=log_sb[:], in_=log_ps[:])
        nc.sync.dma_start(out=logits[:], in_=log_sb[:])

    nc.compile()
    return nc


def get_nc():
    if "k" not in _CACHE:
        _CACHE["k"] = _build_kernel()
    return _CACHE["k"]


def _prepare_in_maps(enc_hidden, query_hidden, num_pairs, q_w, q_b, k_w, out_w, out_b):
    import ml_dtypes
    fp8 = ml_dtypes.float8_e4m3

    L = min(2 * int(num_pairs), T - 3)
    n_valid = max(0, min(L, M))
    start = max(0, L - M)

    qw32 = np.ascontiguousarray(
        np.asarray(q_w, dtype=np.float32)
        .reshape(HC, 128, H).transpose(1, 0, 2).reshape(128, HC * H)
    )
    kwt32 = np.ascontiguousarray(
        np.ascontiguousarray(np.asarray(k_w, dtype=np.float32).T)
        .reshape(HC, 128, H).transpose(1, 0, 2).reshape(128, HC * H)
    )
    qb = np.ascontiguousarray(q_b, dtype=np.float32)
    bf16 = ml_dtypes.bfloat16
    qw16 = qw32.astype(bf16)
    kwt16 = kwt32.astype(bf16)
    qb16 = qb.astype(bf16)
    oww = np.ascontiguousarray(
        np.asarray(out_w, dtype=np.float32)
        .reshape(HC, 128, VOCAB).transpose(1, 0, 2).reshape(128, HC * VOCAB)
    )
    ob = np.ascontiguousarray(out_b, dtype=np.float32)

    in_maps = []
    for core in range(NCORES):
        b0 = core * BP
        sl = np.asarray(enc_hidden[b0:b0 + BP, start:start + n_valid, :], dtype=np.float32)
        if n_valid < M:
            pad = np.zeros((BP, M, H), dtype=np.float32)
            pad[:, :n_valid, :] = sl
            sl = pad
        else:
            sl = np.ascontiguousarray(sl)
        # encT8[j, 32b+hr, (hb, s)] = sl[b, j*S+s, 32*hb+hr]
        encT8 = np.ascontiguousarray(
            sl.reshape(BP, NCHUNK, S, H // 32, 32).transpose(1, 0, 4, 3, 2)
            .reshape(NCHUNK, 128, (H // 32) * S)
        ).astype(fp8)
        in_maps.append({
            "encT8": encT8,
            "enc32": sl,
            "query": np.ascontiguousarray(query_hidden[b0:b0 + BP, :], dtype=np.float32),
            "qw16": qw16,
            "kwt16": kwt16,
            "qb16": qb16,
            "qw32": qw32,
            "kwt32": kwt32,
            "qb": qb,
            "oww": oww,
            "ob": ob,
        })
    return in_maps


def kernel(enc_hidden, query_hidden, num_pairs, q_w, q_b, k_w, k_b, out_w, out_b,
           **run_kwargs):
    """Full-input entry point: shards across 8 NeuronCores, returns (B, VOCAB).

    k_b is accepted (to match the reference signature) but unused: it shifts
    every attention score by the same per-batch constant, which affects
    neither the top-k selection nor the softmax probabilities.
    """
    enc_hidden = np.asarray(enc_hidden)
    query_hidden = np.asarray(query_hidden)
    nc = get_nc()
    in_maps = _prepare_in_maps(
        enc_hidden, query_hidden, num_pairs, q_w, q_b, k_w, out_w, out_b
    )
    res = run_bass_kernel_spmd(nc, in_maps, core_ids=list(range(NCORES)), **run_kwargs)
    out = np.concatenate([res.results[c]["logits"] for c in range(NCORES)], axis=0)
    kernel.last_results = res
    return out
